# revision 5
# baseline (speedup 1.0000x reference)
"""DynamicConv2d (moe_routing) Trainium2 Bass kernel.

Full-input contract: kernel(**inputs) -> np.ndarray [1, 512, 56, 56].

Sharding: 64 conv output channels per core across 8 cores; hash tables +
active-mask computation replicated on every core (the mask needs global
channel ranks, and replicating the small hash matmul avoids a collective);
outputs gathered on host along the channel dim.

Math on device (per core):
  1. conv y_raw[o, s] for its 64 channels via 9 shifted matmuls x 2
     input-channel chunks accumulated in PSUM (float32r).
  2. LSH routing: proj_w = rm_w @ w_flat^T (matmul), bits = proj > 0,
     signature = bits^T @ powers (matmul), same for the query side using
     the *sum* of x over space (positive scale of the mean keeps signs),
     match -> hist -> exact stable-top-k mask via global rank logic.
  3. BN (training stats) + mask + ReLU folded into a per-channel affine:
     out = relu(scale * y_raw + shift) with scale = m*gamma/sqrt(var+eps),
     shift = beta - mean*scale (inactive channels: scale=0, shift=beta).
"""

import numpy as np
from contextlib import ExitStack

import concourse.bass as bass
import concourse.mybir as mybir
import concourse.tile as tile
from concourse import bacc
from concourse.bass_utils import run_bass_kernel_spmd

F32 = mybir.dt.float32
F32R = mybir.dt.float32r
ALU = mybir.AluOpType
ACT = mybir.ActivationFunctionType

N_CORES = 8
O, C, KK, H, W = 512, 256, 3, 56, 56
OC = O // N_CORES          # 64 out channels per core
S = H * W                  # 3136
HP = H + 2                 # 58 padded
T, HASH = 10, 8
TH = T * HASH              # 80
D = C * KK * KK            # 2304
KD = D // 128              # 18 hash contraction chunks
NCH = 7                    # spatial chunks
CH = S // NCH              # 448 columns per PSUM chunk (8 rows of 56)
SIZE_LIMIT = O // 2        # 256
EPS = 1e-3

_CACHE = {}


def _emit(nc):
    xin = nc.dram_tensor("xin", [C, HP, HP], F32R, kind="ExternalInput").ap()
    wconv = nc.dram_tensor("wconv", [128, 2, 9, OC], F32R, kind="ExternalInput").ap()
    whash = nc.dram_tensor("whash", [128, KD, O], F32R, kind="ExternalInput").ap()
    rmt = nc.dram_tensor("rmt", [128, KD, TH], F32R, kind="ExternalInput").ap()
    rqt = nc.dram_tensor("rqt", [128, 2, TH], F32, kind="ExternalInput").ap()
    sigw = nc.dram_tensor("sigw", [TH, T], F32, kind="ExternalInput").ap()
    mlt = nc.dram_tensor("mlt", [128, 4, O], F32, kind="ExternalInput").ap()
    selm = nc.dram_tensor("selm", [128, 4, OC], F32, kind="ExternalInput").ap()
    gamma = nc.dram_tensor("gamma", [OC, 1], F32, kind="ExternalInput").ap()
    beta = nc.dram_tensor("beta", [OC, 1], F32, kind="ExternalInput").ap()
    yout = nc.dram_tensor("yout", [OC, S], F32, kind="ExternalOutput").ap()

    with tile.TileContext(nc) as tc, ExitStack() as ctx:
        consts = ctx.enter_context(tc.tile_pool(name="consts", bufs=1))
        work = ctx.enter_context(tc.tile_pool(name="work", bufs=1))
        scr = ctx.enter_context(tc.tile_pool(name="scr", bufs=2))
        pconv = ctx.enter_context(tc.tile_pool(name="pconv", bufs=3, space="PSUM"))
        psm = ctx.enter_context(tc.tile_pool(name="psm", bufs=3, space="PSUM"))

        # ---- constant / input loads ----
        wconv_sb = consts.tile([128, 2, 9, OC], F32R)
        nc.sync.dma_start(out=wconv_sb, in_=wconv)
        whash_sb = consts.tile([128, KD, O], F32R)
        nc.sync.dma_start(out=whash_sb, in_=whash)
        rmt_sb = consts.tile([128, KD, TH], F32R)
        nc.sync.dma_start(out=rmt_sb, in_=rmt)
        rqt_sb = consts.tile([128, 2, TH], F32)
        nc.sync.dma_start(out=rqt_sb, in_=rqt)
        sigw_sb = consts.tile([TH, T], F32)
        nc.sync.dma_start(out=sigw_sb, in_=sigw)
        mlt_sb = consts.tile([128, 4, O], F32)
        nc.sync.dma_start(out=mlt_sb, in_=mlt)
        selm_sb = consts.tile([128, 4, OC], F32)
        nc.sync.dma_start(out=selm_sb, in_=selm)
        gamma_sb = consts.tile([OC, 1], F32)
        nc.sync.dma_start(out=gamma_sb, in_=gamma)
        beta_sb = consts.tile([OC, 1], F32)
        nc.sync.dma_start(out=beta_sb, in_=beta)

        eps_sb = consts.tile([OC, 1], F32)
        nc.vector.memset(eps_sb, EPS)
        ones10_sb = consts.tile([T, 1], F32)
        nc.vector.memset(ones10_sb, 1.0)
        onesbc_sb = consts.tile([T, 128], F32)
        nc.vector.memset(onesbc_sb, 1.0)

        # padded input (padding done host-side), 2 channel chunks of 128
        xpad = []
        for kc in range(2):
            xp = consts.tile([128, HP, HP], F32R, tag=f"xpad{kc}")
            nc.sync.dma_start(out=xp, in_=xin[kc * 128 : (kc + 1) * 128])
            xpad.append(xp)

        # ---- hash routing chain ----
        # proj_w [TH, O] = sum_d rm[th, d] * w_flat[o, d]
        projw_ps = psm.tile([TH, O], F32, tag="sp")
        for kd in range(KD):
            nc.tensor.matmul(
                projw_ps,
                lhsT=rmt_sb[:, kd, :],
                rhs=whash_sb[:, kd, :],
                start=(kd == 0),
                stop=(kd == KD - 1),
            )
        bits_w = work.tile([TH, O], F32)
        nc.vector.tensor_scalar(bits_w, projw_ps, 0.0, None, ALU.is_gt)

        # sig_w [T, O] = sigw^T @ bits
        sigw_ps = psm.tile([128, O], F32, tag="sp")
        nc.tensor.matmul(
            sigw_ps[:T, :],
            lhsT=sigw_sb,
            rhs=bits_w,
            start=True,
            stop=True,
        )

        # query: channel sums of x (positive scaling of mean keeps hash signs)
        qsum_sb = work.tile([128, 2], F32)
        for kc in range(2):
            nc.vector.tensor_reduce(
                out=qsum_sb[:, kc : kc + 1],
                in_=xpad[kc].bitcast(F32),
                axis=mybir.AxisListType.XY,
                op=ALU.add,
            )
        projq_ps = psm.tile([TH, 1], F32, tag="sp")
        for kc in range(2):
            nc.tensor.matmul(
                projq_ps,
                lhsT=rqt_sb[:, kc, :],
                rhs=qsum_sb[:, kc : kc + 1],
                start=(kc == 0),
                stop=(kc == 1),
            )
        bits_q = work.tile([TH, 1], F32)
        nc.vector.tensor_scalar(bits_q, projq_ps, 0.0, None, ALU.is_gt)
        sigq_ps = psm.tile([T, 1], F32, tag="sp")
        nc.tensor.matmul(sigq_ps, lhsT=sigw_sb, rhs=bits_q, start=True, stop=True)
        sigq_sb = work.tile([T, 1], F32)
        nc.vector.tensor_copy(sigq_sb, sigq_ps)

        # match [T, O] then per-channel table-collision counts
        match_sb = work.tile([T, O], F32)
        nc.vector.tensor_scalar(match_sb, sigw_ps[:T, :], sigq_sb, None, ALU.is_equal)

        # hist in partition orientation: histp[:, j] = match[:, 128j:128j+128]^T @ 1
        histp_ps = psm.tile([128, 4], F32, tag="sp")
        for j in range(4):
            nc.tensor.matmul(
                histp_ps[:, j : j + 1],
                lhsT=match_sb[:, j * 128 : (j + 1) * 128],
                rhs=ones10_sb,
                start=True,
                stop=True,
            )
        histp_sb = work.tile([128, 4], F32)
        nc.vector.tensor_copy(histp_sb, histp_ps)

        # hist broadcast along partitions: histbc[p, o] = hist[o]
        histbc_ps = psm.tile([128, O], F32, tag="sp")
        nc.tensor.matmul(
            histbc_ps,
            lhsT=onesbc_sb,
            rhs=match_sb,
            start=True,
            stop=True,
        )
        histbc_sb = work.tile([128, O], F32)
        nc.vector.tensor_copy(histbc_sb, histbc_ps)

        # exact stable top-k rank: G[o] = #{o': hist>hist[o]} + #{o'<o: hist==hist[o]}
        geq_sb = work.tile([128, 4], F32)
        ggt_sb = work.tile([128, 4], F32)
        for j in range(4):
            s1 = scr.tile([128, O], F32, tag="scratch")
            nc.vector.scalar_tensor_tensor(
                out=s1,
                in0=histbc_sb,
                scalar=histp_sb[:, j : j + 1],
                in1=mlt_sb[:, j, :],
                op0=ALU.is_equal,
                op1=ALU.mult,
                accum_out=geq_sb[:, j : j + 1],
            )
            s2 = scr.tile([128, O], F32, tag="scratch")
            nc.vector.tensor_scalar(
                s2,
                histbc_sb,
                histp_sb[:, j : j + 1],
                None,
                ALU.is_gt,
                op1=ALU.add,
                accum_out=ggt_sb[:, j : j + 1],
            )
        g_sb = work.tile([128, 4], F32)
        nc.vector.tensor_tensor(g_sb, geq_sb, ggt_sb, ALU.add)
        gok_sb = work.tile([128, 4], F32)
        nc.vector.tensor_scalar(gok_sb, g_sb, SIZE_LIMIT - 0.5, None, ALU.is_lt)
        act_sb = work.tile([128, 4], F32)
        nc.vector.scalar_tensor_tensor(
            out=act_sb,
            in0=histp_sb,
            scalar=0.0,
            in1=gok_sb,
            op0=ALU.is_gt,
            op1=ALU.mult,
        )

        # this core's 64-channel mask: mask = selm^T @ active
        mask_ps = psm.tile([OC, 1], F32, tag="sp")
        for j in range(4):
            nc.tensor.matmul(
                mask_ps,
                lhsT=selm_sb[:, j, :],
                rhs=act_sb[:, j : j + 1],
                start=(j == 0),
                stop=(j == 3),
            )
        mask_sb = work.tile([OC, 1], F32)
        nc.vector.tensor_copy(mask_sb, mask_ps)

        # ---- conv: 7 spatial chunks x (2 c-chunks x 9 taps) matmuls ----
        yraw_sb = work.tile([OC, S], F32)
        stats_sb = work.tile([OC, NCH, 6], F32)
        for n in range(NCH):
            acc = pconv.tile([OC, CH], F32, tag="acc")
            i0 = 8 * n
            for kc in range(2):
                for t in range(9):
                    ky, kx = t // 3, t % 3
                    nc.tensor.matmul(
                        acc,
                        lhsT=wconv_sb[:, kc, t, :],
                        rhs=xpad[kc][:, ky + i0 : ky + i0 + 8, kx : kx + W],
                        start=(kc == 0 and t == 0),
                        stop=(kc == 1 and t == 8),
                    )
            nc.vector.bn_stats(out=stats_sb[:, n, :], in_=acc)
            nc.vector.tensor_copy(yraw_sb[:, n * CH : (n + 1) * CH], acc)

        # ---- BN scale/shift + mask + ReLU ----
        mv_sb = work.tile([OC, 2], F32)
        nc.vector.bn_aggr(out=mv_sb, in_=stats_sb.rearrange("p a b -> p (a b)"))
        std_sb = work.tile([OC, 1], F32)
        nc.scalar.activation(std_sb, mv_sb[:, 1:2], ACT.Sqrt, bias=eps_sb)
        rstd_sb = work.tile([OC, 1], F32)
        nc.vector.reciprocal(rstd_sb, std_sb)
        scale0_sb = work.tile([OC, 1], F32)
        nc.vector.tensor_tensor(scale0_sb, gamma_sb, rstd_sb, ALU.mult)
        scale_sb = work.tile([OC, 1], F32)
        nc.vector.tensor_tensor(scale_sb, scale0_sb, mask_sb, ALU.mult)
        msc_sb = work.tile([OC, 1], F32)
        nc.vector.tensor_tensor(msc_sb, mv_sb[:, 0:1], scale_sb, ALU.mult)
        shift_sb = work.tile([OC, 1], F32)
        nc.vector.tensor_tensor(shift_sb, beta_sb, msc_sb, ALU.subtract)

        yfin_sb = work.tile([OC, S], F32)
        nc.scalar.activation(yfin_sb, yraw_sb, ACT.Relu, bias=shift_sb, scale=scale_sb)
        nc.sync.dma_start(out=yout, in_=yfin_sb)

    return nc


def build_nc():
    if "nc" not in _CACHE:
        nc = bacc.Bacc("TRN2", target_bir_lowering=False, debug=False)
        _emit(nc)
        nc.compile()
        _CACHE["nc"] = nc
    return _CACHE["nc"]


def _trunc22(a):
    u = np.ascontiguousarray(a, np.float32).view(np.uint32) & np.uint32(0xFFFFFC00)
    return u.view(np.float32)


def make_in_maps(x, whole_w, rm_w, rm_q, bn_gamma, bn_beta):
    x = np.asarray(x, np.float32)
    whole_w = np.asarray(whole_w, np.float32)
    rm_w = np.asarray(rm_w, np.float32)
    rm_q = np.asarray(rm_q, np.float32)
    bn_gamma = np.asarray(bn_gamma, np.float32)
    bn_beta = np.asarray(bn_beta, np.float32)

    x0 = np.zeros((C, HP, HP), np.float32)
    x0[:, 1 : HP - 1, 1 : HP - 1] = x[0]
    wc9 = whole_w.reshape(O, C, 9)
    w_flat = whole_w.reshape(O, D)
    whash_a = np.ascontiguousarray(
        w_flat.T.reshape(KD, 128, O).transpose(1, 0, 2)
    )
    rmt_a = np.ascontiguousarray(
        rm_w.reshape(TH, D).T.reshape(KD, 128, TH).transpose(1, 0, 2)
    )
    rqt_a = np.ascontiguousarray(
        rm_q.reshape(TH, C).T.reshape(2, 128, TH).transpose(1, 0, 2)
    )
    sigw_a = np.zeros((TH, T), np.float32)
    for t in range(T):
        for h in range(HASH):
            sigw_a[t * HASH + h, t] = float(2 ** (HASH - 1 - h))
    p_idx = np.arange(128)[:, None, None]
    j_idx = np.arange(4)[None, :, None]
    o_idx = np.arange(O)[None, None, :]
    mlt_a = (o_idx < 128 * j_idx + p_idx).astype(np.float32)
    in_maps = []
    for core in range(N_CORES):
        o0 = core * OC
        wconv_a = np.ascontiguousarray(
            wc9[o0 : o0 + OC].reshape(OC, 2, 128, 9).transpose(2, 1, 3, 0)
        )
        m_idx = np.arange(OC)[None, None, :]
        selm_a = (128 * j_idx + p_idx == o0 + m_idx).astype(np.float32)
        in_maps.append(
            {
                "xin": _trunc22(x0),
                "wconv": _trunc22(wconv_a),
                "whash": _trunc22(whash_a),
                "rmt": _trunc22(rmt_a),
                "rqt": rqt_a,
                "sigw": sigw_a,
                "mlt": mlt_a,
                "selm": np.ascontiguousarray(selm_a),
                "gamma": np.ascontiguousarray(bn_gamma[o0 : o0 + OC, None]),
                "beta": np.ascontiguousarray(bn_beta[o0 : o0 + OC, None]),
            }
        )
    return in_maps


def kernel(x, whole_w, rm_w, rm_q, bn_gamma, bn_beta):
    nc = build_nc()
    in_maps = make_in_maps(x, whole_w, rm_w, rm_q, bn_gamma, bn_beta)
    res = run_bass_kernel_spmd(nc, in_maps, list(range(N_CORES)))
    y = np.concatenate([r["yout"] for r in res.results], axis=0)
    return y.reshape(1, O, H, W).astype(np.float32)
